# revision 45
# baseline (speedup 1.0000x reference)
"""Distributed Bass kernel for nn_Attention_64269890617453 on 8 TRN2 NeuronCores.

Math (reference):
    q = relu(x@Wq+bq); k = relu(x@Wk+bk); v = relu(x@Wv+bv)    [8192,128]
    adj = softmax(leaky_relu(q @ k.T, 0.2), axis=1)             [8192,8192]
    out = adj @ v                                               [8192,128]

Exact simplifications:
  - q,k >= 0 (relu outputs) so leaky_relu is the identity on q@k.T.
  - scores are ~7 +/- 3 (max ~24): softmax needs no max-subtraction in fp32.

Sharding: q rows split across 8 cores (1024 each); k/v computed redundantly
per core from the full x (collectives cost more than the redundant compute).

v10 design (~103.5-105us fast-clock, vs v3's 103.8; the chip runs whole
NEFFs in one of two DVFS states ~19% apart, uncontrollable from here, and
per-core input-DMA luck adds +-1us to the worst core):
  - steady state: 64 blocks at the ACT pace of 1.114us/block (exp
    [128,1024] back-to-back, measured gap sum < 0.2us) with the PE floor at
    ~1.105 (S 1024c + AV 1032c + proj 512c at 2.37GHz) -- the two engines
    are co-bound within ~1%, and this is the architectural floor.
  - fill: exp(0) at ~18.2us (v3: 19.4).  piece0+wk are first on their
    queues and the chunk-0 k chain is FIRST in the PE stream, overlapping
    the xq transfer; xq rides as four [128,512] quarter transfers split
    across sync/gpsimd; the two q psum halves and the two qT halves are
    separate tiles (a shared tile serializes relu-0 against the h1 matmuls
    via whole-tile WAR); chunk-0's kT is additionally split per tk-block so
    S(0) waits one 128-row relu, not four.  Input DMA is AGGREGATE
    HBM-bound (8 cores pull the same ~1MB window at ~1.3TB/s total), so
    trigger parallelism matters less than keeping pieces 2-5 (gpsimd,
    dummy-memset spacers) out of the critical window.
  - epilogue: per-av single-engine multiplies (av2/av0 DVE, av1 ACT) and
    the three out-DMAs on three different queues (sync/scalar/gpsimd);
    av2-first everywhere.  Out-DMA completion semaphores lag ~2.8us; the
    NRT-injected NEFF wrapper adds ~7.2us of entry barriers and ~7us of
    per-semaphore teardown clears -- all three are runtime-fixed (not in
    the walrus-emitted program; --max-sem-num etc. change nothing).
  - negative results worth keeping (v4-v8 all measured SLOWER):
    * fp8 anywhere is numerically dead: softmax amplifies absolute S error
      (e4m3 q/k -> 7e-2 final err; even v-only fp8 -> 3.9e-2; gate 2e-2).
    * splitting exp ACT/DVE (trailing-cols int16 Schraudolph, numerically
      fine at ~1.1e-2) settles the cadence at ~1.18us/block regardless of
      scheduling (same-iteration, one-block-ahead, separate output tiles):
      the extra cross-engine waits + legalizer NoOp carriers on the Tensor
      queue cost more than the ~100ns/block of ACT relief.
    * 4-way chunk-proj splits (k-relu halves, v-add/v-max on separate
      iterations) starve the single pj psum bank's kp->vp ping-pong.
    * all-gather k/v via collective_compute can't beat the redundant
      compute: the gather delivers all-at-once (chunk 1 is needed ~4us
      after exp(0)) and gathered k/v reads cost the same HBM bytes as x.

Toolchain workarounds (unchanged): _legalize_waits hoists excess sem-waits
onto NoOp carriers; patched TileContext exit splits drain waits and replaces
the dma_reset + barrier exit with one spanning sem range-clear.  gpsimd
cannot access PSUM (BIR verifier) and custom-DVE ops don't codegen in this
toolchain ("ISA wrong length").  DMA triggers only on sync/scalar/gpsimd
queues; a trigger costs ~0.65us of queue time.
"""

import sys
import time

import numpy as np

try:
    import concourse.bass as bass  # noqa: F401
except ImportError:  # pragma: no cover - fallback when PYTHONPATH is bare
    sys.path.insert(0, "/opt/trn_rl_repo")

import ml_dtypes

import concourse.bass as bass
import concourse.mybir as mybir
import concourse.tile as tile
from concourse.bass_utils import run_bass_kernel_spmd

N, IN, OUT = 8192, 256, 128
NCORES = 8
ROWS = N // NCORES  # 1024 q rows per core
BF = mybir.dt.bfloat16
F32 = mybir.dt.float32
I16 = mybir.dt.int16
BLK = 128  # tk block
NBLK = N // BLK  # 64
VW = OUT + 1  # 129: v block width incl. ones column

# one-op int16 Schraudolph: i16 bits of bf16(e^s) = s*2^7*log2(e) + 2^7*(127-c)
# (s >= 0 always: q,k are relu outputs, so no sign handling needed; max s ~23
# keeps the i16 under 21k).  c=0.043 centers the sawtooth error (+-3.5% max).
EXP16_C = 0.043
EXP16_A = float(np.float32(2**7 * np.log2(np.e)))
EXP16_B = float(np.float32(2**7 * (127.0 - EXP16_C)))
# exp cols on ACT (261ns fixed + 0.832ns/col = 1006ns); DVE takes the last
# 128 (~350ns).  NOTE the split axis is q-rows: rows 896..1023 of each
# core's 1024 get pure fast-exp (measured end-to-end 8.1e-3 vs gate 2e-2).
# 896 keeps ACT just under the ~1.09us/block PE floor; DVE's worst
# iteration (fast-exp + v-add) lands ~1.05us.
ACT_COLS = 896


def _install_drain_patch():
    """This compiler build caps sync-waits per instruction at 1; the Tile exit
    drain carries one wait per in-flight proc.  Split them across drains."""
    from bass_rust import ScopedClock

    if getattr(tile.TileContext, "_drain_patch_installed", False):
        return

    def _patched(self, tick_clock, wait_clock):
        drain_inst = self.nc.sync.drain()
        wait_clock.add_sem_waits(
            drain_inst.ins, ScopedClock({None: tick_clock.global_clock})
        )
        si = drain_inst.ins.sync_info
        waits = list(si.on_wait)
        last = drain_inst
        if len(waits) > 1:
            si.on_wait = waits[:1]
            for w in waits[1:]:
                extra = self.nc.sync.drain()
                extra.ins.sync_info = mybir.SyncInfo(on_wait=[w], on_update=[])
                last = extra
        assert self.sems is not None
        popped = self.nc._tile_sem_poison_stack.pop()
        assert popped is self._sem_poison
        sems = list(self.sems.allocated().values())
        if sems:
            nums = [s.num if hasattr(s, "num") else s for s in sems]
            span = range(min(nums), max(nums) + 1)
            # The drain chain above observed every proc's final tick, so all
            # sem consumers have retired; a single sem hop orders the clear
            # after it -- no all-engine barrier butterfly needed.
            gate = self.nc._state.alloc_semaphore()
            last.then_inc(gate, 1)
            self.nc.gpsimd.wait_ge(gate, 1)
            self.nc.gpsimd.sem_clear(span)
            self.nc.gpsimd.sem_clear(range(gate.num, gate.num + 1) if hasattr(gate, "num") else gate)

    tile.TileContext._drain_and_barrier = _patched
    tile.TileContext._drain_patch_installed = True


_CAP1_OPCODES = {"DMACopy", "Drain", "EventSemaphore", "TriggeredCopy"}
_DEFAULT_CAP = 1


def _legalize_waits(nc):
    """This toolchain encodes at most 1 sem-wait on queue/CTRL instructions
    (DMACopy, Drain) and ~2 on compute-engine instructions; Tile emits more.
    Hoist excess waits onto NoOp carriers on the same engine immediately
    before the overloaded instruction."""
    n_fix = 0
    for fn in nc.m.functions:
        for blk in fn.blocks:
            new_insts = []
            for inst in blk.instructions:
                si = inst.sync_info
                waits = list(si.on_wait) if si is not None else []
                cap = 1 if str(inst.opcode) in _CAP1_OPCODES else _DEFAULT_CAP
                if len(waits) > cap:
                    keep = waits[:cap]
                    rest = waits[cap:]
                    for k, w in enumerate(rest):
                        nop = mybir.InstNoOp(
                            name=f"{inst.name}-w{k}", ins=[], outs=[]
                        )
                        nop.engine = inst.engine
                        nop.sync_info = mybir.SyncInfo(on_wait=[w], on_update=[])
                        new_insts.append(nop)
                    inst.sync_info = mybir.SyncInfo(
                        on_wait=keep, on_update=list(si.on_update)
                    )
                    n_fix += 1
                new_insts.append(inst)
            blk.instructions = new_insts
    return n_fix


def build_bass():
    _install_drain_patch()
    nc = bass.Bass()
    xT = nc.dram_tensor("xT", [IN, N], BF, kind="ExternalInput")
    xTq = nc.dram_tensor("xTq", [IN, ROWS], BF, kind="ExternalInput")
    # Wall = Wq|Wk|Wv (two 128-row K-blocks each, side by side) followed by
    # the host-broadcast v-bias plane (every row = bv|bv|bv|bv).  The bias
    # plane is a full 128-partition block because 1-partition DMA completion
    # semaphores fire ~20us late on this runtime.
    Wall = nc.dram_tensor("Wall", [128, 3 * IN + 4 * OUT], BF, kind="ExternalInput")
    Ball = nc.dram_tensor("Ball", [128, 2], F32, kind="ExternalInput")
    out_d = nc.dram_tensor("out", [ROWS, OUT], F32, kind="ExternalOutput")

    AT = mybir.ActivationFunctionType
    OP = mybir.AluOpType

    NCHUNK = 16          # 512-token chunks
    BPC = 4              # tk blocks per chunk

    with tile.TileContext(nc) as tc:
        with (
            tc.tile_pool(name="persist", bufs=1) as persist,
            tc.tile_pool(name="wpool", bufs=1) as wpool,
            tc.tile_pool(name="pp", bufs=4) as pp,
            tc.tile_pool(name="ep", bufs=8) as ep,
            tc.tile_pool(name="pj", bufs=1, space="PSUM") as pj,
            tc.tile_pool(name="sp", bufs=2, space="PSUM") as sp,
            tc.tile_pool(name="avp", bufs=1, space="PSUM") as avp,
        ):
            # ---- persistent SBUF
            # x split into piece tiles so early chunks unblock as soon as
            # their piece lands (tile-granular deps; no subtile tracking).
            # each piece holds BOTH 128-row halves of xT side by side and is
            # filled by ONE 3D DMA -- fewer DMA rings means less per-queue
            # teardown churn in the walrus-generated postamble.
            PIECES = [(0, 512), (512, 1024), (1024, 2048), (2048, 4096), (4096, 6144), (6144, 8192)]
            xP = [persist.tile([128, 2 * (e - s0)], BF, tag=f"xP{i}", name=f"xP{i}")
                  for i, (s0, e) in enumerate(PIECES)]

            def xview(half, lo, hi):
                for i, (s0, e) in enumerate(PIECES):
                    if s0 <= lo and hi <= e:
                        w = e - s0
                        return xP[i][:, half * w + lo - s0 : half * w + hi - s0]
                raise AssertionError((lo, hi))

            def dma_piece(eng, i):
                s0, e = PIECES[i]
                dst = xP[i][:].rearrange("p (h c) -> p h c", h=2)
                src = xT[:, s0:e].rearrange("(h p) c -> p h c", p=128)
                eng.dma_start(dst, src)

            # xq in four [128,512] quarter tiles (two per 128-feature half):
            # the first q matmuls need only the 'a' quarters, so they start
            # ~1us into the xq transfer instead of after all 512KB
            xq0a = persist.tile([128, 512], BF, tag="xq0a")
            xq0b = persist.tile([128, 512], BF, tag="xq0b")
            xq1a = persist.tile([128, 512], BF, tag="xq1a")
            xq1b = persist.tile([128, 512], BF, tag="xq1b")
            kTs = [persist.tile([128, 512], BF, tag=f"kT{j}", name=f"kT{j}") for j in range(NCHUNK)]
            # chunk 0's kT additionally split per tk-block: S(0) then waits
            # only the first 128-row relu instead of the whole 512 (the
            # k-relu sits on the critical fill path to the first exp)
            kT0b = [persist.tile([128, 128], BF, tag=f"kT0b{t}", name=f"kT0b{t}")
                    for t in range(BPC)]
            vSs = [persist.tile([128, BPC * VW], BF, tag=f"vS{j}", name=f"vS{j}") for j in range(NCHUNK)]
            # qT in two half tiles: S(b)'s first matmul reads only half 0, so
            # it can issue after relu-0 instead of waiting for both q relus
            # (tile deps are whole-tile)
            qTh = [persist.tile([128, 512], BF, tag=f"qT{h}", name=f"qT{h}")
                   for h in range(2)]
            warm = persist.tile([128, 512], BF, tag="warm")

            wall = wpool.tile([128, 3 * IN + 4 * OUT], BF, tag="wall")
            ball = wpool.tile([128, 2], F32, tag="ball")
            bvb4 = wall[:, 3 * IN : 3 * IN + 4 * OUT]
            wq, wk, wv = wall[:, 0:IN], wall[:, IN : 2 * IN], wall[:, 2 * IN : 3 * IN]
            bq_s, bk_s = ball[:, 0:1], ball[:, 1:2]

            # ---- input DMA: only sync/scalar/gpsimd queues can trigger DMA.
            # The input path is AGGREGATE HBM-bandwidth bound (all 8 cores
            # pull the same data; the first-wave ~0.8MB/core window drains at
            # ~1.3TB/s total), so the wave is ordered by NEED: piece0+wk feed
            # the k0 projection (which overlaps the bigger xq wait), the xq
            # 'a' quarters feed the first q matmuls, 'b' quarters follow.
            dma_piece(nc.sync, 0)
            nc.sync.dma_start(xq0a[:], xTq[0:128, 0:512])
            nc.sync.dma_start(xq0b[:], xTq[0:128, 512:1024])
            nc.sync.dma_start(ball[:], Ball[:])

            # gpsimd: warm tile memset first (feeds the PE warmup below, and
            # must not wait behind the triggers' 0.7us each).
            nc.gpsimd.memset(warm[:], 0.0)
            nc.gpsimd.dma_start(wall[:, 0:IN], Wall[:, 0:IN])
            nc.gpsimd.dma_start(xq1a[:], xTq[128:256, 0:512])
            nc.gpsimd.dma_start(xq1b[:], xTq[128:256, 512:1024])

            # scalar (= ACT queue): wk first (k0 chain needs it ~12us), then
            # a ~1.3us memset spacer before wv+piece1 -- neither is needed
            # until ~19.5us (v0 proj / chunk-1 k proj) and together they are
            # 448KB/core (3.6MB aggregate) that otherwise competes with the
            # critical xq window on HBM.  The spacer is sized so the queue
            # still reaches the auto-inserted ACT exp-table load (~1.3us)
            # before the first q-relu needs it.
            delay2 = wpool.tile([128, 1024], F32, tag="delay2")
            nc.scalar.dma_start(wall[:, IN : 2 * IN], Wall[:, IN : 2 * IN])
            nc.scalar.activation(delay2[:, 0:512], warm[:], AT.Copy)
            nc.scalar.activation(delay2[:, 512:1024], warm[:], AT.Copy)
            nc.scalar.dma_start(
                wall[:, 2 * IN : 3 * IN + 4 * OUT], Wall[:, 2 * IN : 3 * IN + 4 * OUT]
            )
            dma_piece(nc.scalar, 1)

            def ones_cols(jlo, jhi):
                for j in range(jlo, jhi):
                    vv = vSs[j][:].rearrange("p (b c) -> p b c", c=VW)
                    nc.gpsimd.memset(vv[:, :, OUT : OUT + 1], 1.0)

            # pieces 2-5 are spaced with dummy-memset timers so their
            # transfers start only after the critical window drains (v4
            # fired pieces 2-3 at ~9-11us and starved xq1/wq until ~17.7us:
            # 27MB of aggregate demand in an ~8MB/7us window).  Deadlines
            # (chunk c's k-proj at ~first_exp + 4.3c us) leave >2us margin.
            delay = wpool.tile([128, 3072], F32, tag="delay")
            ones_cols(0, NCHUNK)
            nc.gpsimd.memset(delay[:], 0.0)  # ~2.5us spacer
            dma_piece(nc.gpsimd, 2)
            nc.gpsimd.memset(delay[:], 0.0)
            dma_piece(nc.gpsimd, 3)
            nc.gpsimd.memset(delay[:], 0.0)
            dma_piece(nc.gpsimd, 4)
            nc.gpsimd.memset(delay[:], 0.0)
            dma_piece(nc.gpsimd, 5)

            # AV accumulators: 8 tq-chunks of [128, 129], 3 per PSUM bank pair
            av0 = avp.tile([128, 3 * VW], F32, tag="av0")
            av1 = avp.tile([128, 3 * VW], F32, tag="av1")
            av2 = avp.tile([128, 2 * VW], F32, tag="av2")
            chunk_map = [
                (av0, 0), (av0, 1), (av0, 2),
                (av1, 0), (av1, 1), (av1, 2),
                (av2, 0), (av2, 1),
            ]

            # PE warm-up burst off the memset tile: HAM clock-gate release +
            # DVFS ramp while the input DMAs are in flight (v3's warmups read
            # the wall tile and sat blocked on its DMA until ~9.7us).  3
            # matmuls measured best; stretching the burst to bridge the whole
            # DMA-wait window (9 matmuls) came back ~0.7us WORSE.
            for wu in range(3):
                nc.tensor.matmul(
                    av0[:, 0:384] if wu % 2 == 0 else av1[:, 0:384],
                    warm[:, 0:128],
                    warm[:, 128:512],
                    start=True, stop=True, skip_group_check=True,
                )

            # chunk-projection work in 2 sub-pieces staggered across the
            # chunk's iterations (the v3 schedule: it keeps TWO iterations
            # between each pj-bank release and the next alloc):
            #   t0: k matmuls + k-relu
            #   t2: v matmuls + v bias add + v max
            def proj_piece(j, t):
                if t == 0:
                    kp = pj.tile([128, 512], F32, tag="pj", name=f"kp_{j}")
                    nc.tensor.matmul(kp[:], wk[:, 0:128], xview(0, j * 512, (j + 1) * 512), start=True, stop=False)
                    nc.tensor.matmul(kp[:], wk[:, 128:256], xview(1, j * 512, (j + 1) * 512), start=False, stop=True)
                    if j == 0:
                        for tt in range(BPC):
                            nc.vector.tensor_scalar(
                                kT0b[tt][:], kp[:, tt * 128 : (tt + 1) * 128],
                                bk_s, 0.0, OP.add, OP.max,
                            )
                    else:
                        nc.vector.tensor_scalar(
                            kTs[j][:], kp[:], bk_s, 0.0, OP.add, OP.max
                        )
                elif t == 2:
                    vp = pj.tile([128, 512], F32, tag="pj", name=f"vp_{j}")
                    # all x0-side matmuls first, then all x1-side: each group
                    # shares ONE moving AP (a wv half) with rotating
                    # stationary slices -- the AV-stream pattern, which hides
                    # LDWEIGHTS.  Per-slice accumulation order is unchanged.
                    for vt in range(BPC):
                        lo = j * 512 + vt * 128
                        ds = slice(vt * 128, (vt + 1) * 128)
                        nc.tensor.matmul(
                            vp[:, ds], xview(0, lo, lo + 128), wv[:, 0:128],
                            start=(vt == 0), stop=False, skip_group_check=True,
                        )
                    for vt in range(BPC):
                        lo = j * 512 + vt * 128
                        ds = slice(vt * 128, (vt + 1) * 128)
                        nc.tensor.matmul(
                            vp[:, ds], xview(1, lo, lo + 128), wv[:, 128:256],
                            start=False, stop=(vt == BPC - 1), skip_group_check=True,
                        )
                    nc.vector.tensor_tensor(vp[:], vp[:], bvb4[:], mybir.AluOpType.add)
                    vview = vSs[j][:].rearrange("p (b c) -> p b c", c=VW)
                    vpview = vp[:].rearrange("p (b c) -> p b c", c=128)
                    nc.vector.tensor_scalar_max(vview[:, :, 0:OUT], vpview[:], 0.0)

            stiles = {}

            def emit_S(b):
                j, t = divmod(b, BPC)
                s = sp.tile([128, ROWS], F32, tag="s", name=f"s_{b}")
                lhs = kT0b[t][:] if j == 0 else kTs[j][:, t * 128 : (t + 1) * 128]
                nc.tensor.matmul(s[:, 0:512], lhs, qTh[0][:], start=True, stop=True)
                nc.tensor.matmul(
                    s[:, 512:1024], lhs, qTh[1][:], start=True, stop=True
                )
                stiles[b] = s

            ptiles = {}

            # exp is 100% ACT table-exp, as in v3.  Every DVE-assisted split
            # tried (v4-v8: trailing-cols Schraudolph, one-block-ahead
            # emission, separate output tiles) settled the steady cadence at
            # ~1.18us/block vs all-ACT's ~1.10: the extra cross-engine waits
            # and their NoOp carriers on the Tensor queue cost more than the
            # ~100ns/block of ACT relief buys.
            def emit_exp(b):
                s = stiles.pop(b)
                p = pp.tile([128, ROWS], BF, tag="p", name=f"p_{b}")
                ptiles[b] = p
                nc.scalar.activation(p[:], s[:], AT.Exp)

            def emit_AV(b):
                j, t = divmod(b, BPC)
                p = ptiles.pop(b)
                vblk = vSs[j][:, t * VW : (t + 1) * VW]
                # final block writes av2's chunks FIRST: the epilogue
                # processes av2 first, so its reciprocal unblocks ~0.3us
                # sooner (accumulation order within a block is free)
                order = reversed(range(8)) if b == NBLK - 1 else range(8)
                for c in order:
                    av, sub = chunk_map[c]
                    nc.tensor.matmul(
                        av[:, sub * VW : (sub + 1) * VW],
                        p[:, c * 128 : (c + 1) * 128],
                        vblk,
                        start=(b == 0 and sub == 0),
                        stop=(b == NBLK - 1),
                        skip_group_check=True,
                    )

            # Pipeline: the PE stream per iteration is [S(b+1); AV(b-1)],
            # with exp(b) split ACT/DVE in between.  S runs one block ahead
            # and AV one behind so the PE never waits on the exp of the block
            # it just produced -- keeps the tensor engine fed and at full
            # clock.  PSUM accumulation order across blocks is irrelevant
            # (block 0 carries start=, block 63 stop=).
            # ---- qT = relu(Wq.T @ xq + bq) ----
            # q-half 0 (from the xq 'a' quarters) then its relu; the chunk-0
            # k projection (matmuls + both DVE relu halves -- S(0) reads the
            # whole kT0 tile) goes between the two q halves so the k-chain
            # overlaps the 'b' quarters' transfer.  The two q halves use
            # SEPARATE psum tiles: with one shared tile, relu-0 blocks the
            # h1 matmuls on a whole-tile write-after-read for ~2us.
            # q relus ride ACT (idle until exp(0); Relu shares the exp table
            # set so no table reload) -- keeps the DVE queue off the S(0)
            # critical chain.
            # chunk-0 k chain FIRST in the PE stream: piece0 and wk are the
            # first transfers on their queues, so the kp matmuls + DVE relu
            # run while the (bigger) xq quarters are still in flight.
            proj_piece(0, 0)
            for h, (xa, xb) in enumerate(((xq0a, xq1a), (xq0b, xq1b))):
                qp = sp.tile([128, 512], F32, tag="s", name=f"qp{h}")
                nc.tensor.matmul(qp[:], wq[:, 0:128], xa[:],
                                 start=True, stop=False, skip_group_check=True)
                nc.tensor.matmul(qp[:], wq[:, 128:256], xb[:],
                                 start=False, stop=True, skip_group_check=True)
                nc.scalar.activation(qTh[h][:], qp[:], AT.Relu, bias=bq_s)

            # v0 comes AFTER S(0): the first exp only needs q/k0/S0, and v0
            # is needed one block later by AV(0)
            emit_S(0)
            avq = []
            for j in range(NCHUNK):
                for t in range(BPC):
                    b = j * BPC + t
                    if b + 1 < NBLK:
                        emit_S(b + 1)
                    if b == 0:
                        # chunk-0 v chain AFTER S(1) in the PE stream: its
                        # 8 matmuls otherwise delay exp(1) by ~1us during
                        # pipeline fill (AV(0), their consumer, runs a full
                        # block later and is not critical)
                        proj_piece(0, 2)
                    emit_exp(b)
                    # stagger next chunk's projection pieces between blocks;
                    # k at t==0 / v at t==2 gives the DVE relu chain a full
                    # extra block of slack before kT/vS are consumed at the
                    # chunk boundary
                    if j + 1 < NCHUNK:
                        if t == 0:
                            proj_piece(j + 1, 0)
                        elif t == 2:
                            proj_piece(j + 1, 2)
                    avq.append(b)
                    while avq and avq[0] <= b - 1:
                        emit_AV(avq.pop(0))
            for b in avq:
                emit_AV(b)

            # ---- epilogue: divide by the ones-column denominator, DMA out.
            # one strided reciprocal + grouped result tile + one DMA per av
            # accumulator, the three DMAs on three DIFFERENT queues so their
            # trigger cost and completion-semaphore latency overlap.  av2
            # first: the final AV block writes its chunks first, so its
            # reciprocal unblocks soonest.
            # each av's multiplies are SINGLE-engine (av2/av0 on DVE, av1 on
            # ACT) so no av's DMA trigger chains behind another av's ops on a
            # shared engine queue (v5's av1 trigger on the scalar queue sat
            # behind all three ACT copies).  av0 -- the last one ready --
            # rides sync as its SECOND trigger: the gpsimd DMA ring's
            # completion semaphore lags ~1us behind the sync/scalar rings,
            # and av0's completion gates the exit drain.
            for av, nsub, base, eng, mul_eng in (
                (av2, 2, 6, nc.sync, "dve"),
                (av1, 3, 3, nc.scalar, "act"),
                (av0, 3, 0, nc.sync, "dve"),
            ):
                avv = av[:].rearrange("p (b c) -> p b c", c=VW)
                rc = ep.tile([128, nsub], F32, tag=f"rc{base}", name=f"rc_{base}")
                nc.vector.reciprocal(rc[:], avv[:, :, OUT])
                res = ep.tile([128, nsub * OUT], F32, tag=f"res{base}", name=f"res_{base}")
                rview = res[:].rearrange("p (b c) -> p b c", c=OUT)
                for s2 in range(nsub):
                    if mul_eng == "dve":
                        nc.vector.tensor_scalar_mul(
                            rview[:, s2, :], avv[:, s2, 0:OUT], rc[:, s2 : s2 + 1]
                        )
                    else:
                        nc.scalar.activation(
                            rview[:, s2, :], avv[:, s2, 0:OUT], AT.Copy,
                            scale=rc[:, s2 : s2 + 1],
                        )
                dst = out_d[base * 128 : (base + nsub) * 128, :].rearrange(
                    "(b p) c -> p b c", p=128
                )
                eng.dma_start(dst, rview[:])

    _legalize_waits(nc)
    return nc


_NC_CACHE = None


def _get_nc():
    global _NC_CACHE
    if _NC_CACHE is None:
        _NC_CACHE = build_bass()
    return _NC_CACHE


def _prep_inputs(x, Wq, bq, Wk, bk, Wv, bv):
    bf = ml_dtypes.bfloat16
    xT = np.ascontiguousarray(np.asarray(x, np.float32).T).astype(bf)  # [256, 8192]

    def w2(W):  # [256,128] -> [128, 256] with the two 128-row K-blocks side by side
        W = np.asarray(W, np.float32)
        return np.ascontiguousarray(np.concatenate([W[:128], W[128:]], axis=1)).astype(bf)

    base = {
        "xT": xT,
        "Wall": np.ascontiguousarray(
            np.concatenate(
                [
                    w2(Wq), w2(Wk), w2(Wv),
                    np.broadcast_to(
                        np.tile(np.asarray(bv, np.float32), 4)[None, :],
                        (128, 4 * OUT),
                    ).astype(bf),
                ],
                axis=1,
            )
        ),
        "Ball": np.ascontiguousarray(
            np.stack(
                [np.asarray(bq, np.float32), np.asarray(bk, np.float32)], axis=1
            )
        ),
    }
    in_maps = []
    for c in range(NCORES):
        m = dict(base)
        m["xTq"] = np.ascontiguousarray(xT[:, c * ROWS : (c + 1) * ROWS])
        in_maps.append(m)
    return in_maps


def kernel(x, Wq, bq, Wk, bk, Wv, bv):
    nc = _get_nc()
    in_maps = _prep_inputs(x, Wq, bq, Wk, bk, Wv, bv)
    last_err = None
    for attempt in range(3):
        try:
            res = run_bass_kernel_spmd(nc, in_maps, core_ids=list(range(NCORES)))
            break
        except Exception as e:  # transient NRT_EXEC_UNIT_UNRECOVERABLE after a
            last_err = e       # previously crashed run wedges the device once
            if attempt == 2:
                raise
            time.sleep(2)
    return np.concatenate([res.results[c]["out"] for c in range(NCORES)], axis=0)


if __name__ == "__main__":
    rng = np.random.default_rng(0)
    s = 1.0 / np.sqrt(IN)
    x = rng.standard_normal((N, IN), dtype=np.float32)
    args = dict(
        x=x,
        Wq=rng.uniform(-s, s, (IN, OUT)).astype(np.float32),
        bq=rng.uniform(-s, s, OUT).astype(np.float32),
        Wk=rng.uniform(-s, s, (IN, OUT)).astype(np.float32),
        bk=rng.uniform(-s, s, OUT).astype(np.float32),
        Wv=rng.uniform(-s, s, (IN, OUT)).astype(np.float32),
        bv=rng.uniform(-s, s, OUT).astype(np.float32),
    )
    o = kernel(**args)
    q = np.maximum(x @ args["Wq"] + args["bq"], 0)
    k = np.maximum(x @ args["Wk"] + args["bk"], 0)
    v = np.maximum(x @ args["Wv"] + args["bv"], 0)
    S = q @ k.T
    P = np.exp(S - S.max(1, keepdims=True))
    ref = (P / P.sum(1, keepdims=True)) @ v
    print("max rel err:", np.abs(o - ref).max() / np.abs(ref).max())


# revision 46
# speedup vs baseline: 1.1989x; 1.1989x over previous
"""Distributed Bass kernel for nn_Attention_64269890617453 on 8 TRN2 NeuronCores.

Math (reference):
    q = relu(x@Wq+bq); k = relu(x@Wk+bk); v = relu(x@Wv+bv)    [8192,128]
    adj = softmax(leaky_relu(q @ k.T, 0.2), axis=1)             [8192,8192]
    out = adj @ v                                               [8192,128]

Exact simplifications:
  - q,k >= 0 (relu outputs) so leaky_relu is the identity on q@k.T.
  - scores are ~7 +/- 3 (max ~24): softmax needs no max-subtraction in fp32.

Sharding: q rows split across 8 cores (1024 each); k/v computed redundantly
per core from the full x (collectives cost more than the redundant compute).

v10 design (~103.5-105us fast-clock, vs v3's 103.8; the chip runs whole
NEFFs in one of two DVFS states ~19% apart, uncontrollable from here, and
per-core input-DMA luck adds +-1us to the worst core):
  - steady state: 64 blocks at the ACT pace of 1.114us/block (exp
    [128,1024] back-to-back, measured gap sum < 0.2us) with the PE floor at
    ~1.105 (S 1024c + AV 1032c + proj 512c at 2.37GHz) -- the two engines
    are co-bound within ~1%, and this is the architectural floor.
  - fill: exp(0) at ~18.2us (v3: 19.4).  piece0+wk are first on their
    queues and the chunk-0 k chain is FIRST in the PE stream, overlapping
    the xq transfer; xq rides as four [128,512] quarter transfers split
    across sync/gpsimd; the two q psum halves and the two qT halves are
    separate tiles (a shared tile serializes relu-0 against the h1 matmuls
    via whole-tile WAR); chunk-0's kT is additionally split per tk-block so
    S(0) waits one 128-row relu, not four.  Input DMA is AGGREGATE
    HBM-bound (8 cores pull the same ~1MB window at ~1.3TB/s total), so
    trigger parallelism matters less than keeping pieces 2-5 (gpsimd,
    dummy-memset spacers) out of the critical window.
  - epilogue: per-av single-engine multiplies (av2/av0 DVE, av1 ACT) and
    the three out-DMAs on three different queues (sync/scalar/gpsimd);
    av2-first everywhere.  Out-DMA completion semaphores lag ~2.8us; the
    NRT-injected NEFF wrapper adds ~7.2us of entry barriers and ~7us of
    per-semaphore teardown clears -- all three are runtime-fixed (not in
    the walrus-emitted program; --max-sem-num etc. change nothing).
  - negative results worth keeping (v4-v8 all measured SLOWER):
    * fp8 anywhere is numerically dead: softmax amplifies absolute S error
      (e4m3 q/k -> 7e-2 final err; even v-only fp8 -> 3.9e-2; gate 2e-2).
    * splitting exp ACT/DVE (trailing-cols int16 Schraudolph, numerically
      fine at ~1.1e-2) settles the cadence at ~1.18us/block regardless of
      scheduling (same-iteration, one-block-ahead, separate output tiles):
      the extra cross-engine waits + legalizer NoOp carriers on the Tensor
      queue cost more than the ~100ns/block of ACT relief.
    * 4-way chunk-proj splits (k-relu halves, v-add/v-max on separate
      iterations) starve the single pj psum bank's kp->vp ping-pong.
    * all-gather k/v via collective_compute can't beat the redundant
      compute: the gather delivers all-at-once (chunk 1 is needed ~4us
      after exp(0)) and gathered k/v reads cost the same HBM bytes as x.

Toolchain workarounds (unchanged): _legalize_waits hoists excess sem-waits
onto NoOp carriers; patched TileContext exit splits drain waits and replaces
the dma_reset + barrier exit with one spanning sem range-clear.  gpsimd
cannot access PSUM (BIR verifier) and custom-DVE ops don't codegen in this
toolchain ("ISA wrong length").  DMA triggers only on sync/scalar/gpsimd
queues; a trigger costs ~0.65us of queue time.
"""

import sys
import time

import numpy as np

try:
    import concourse.bass as bass  # noqa: F401
except ImportError:  # pragma: no cover - fallback when PYTHONPATH is bare
    sys.path.insert(0, "/opt/trn_rl_repo")

import ml_dtypes

import concourse.bass as bass
import concourse.mybir as mybir
import concourse.tile as tile
from concourse.bass_utils import run_bass_kernel_spmd

N, IN, OUT = 8192, 256, 128
NCORES = 8
ROWS = N // NCORES  # 1024 q rows per core
BF = mybir.dt.bfloat16
F32 = mybir.dt.float32
I16 = mybir.dt.int16
BLK = 128  # tk block
NBLK = N // BLK  # 64
VW = OUT + 1  # 129: v block width incl. ones column

# one-op int16 Schraudolph: i16 bits of bf16(e^s) = s*2^7*log2(e) + 2^7*(127-c)
# (s >= 0 always: q,k are relu outputs, so no sign handling needed; max s ~23
# keeps the i16 under 21k).  c=0.043 centers the sawtooth error (+-3.5% max).
EXP16_C = 0.043
EXP16_A = float(np.float32(2**7 * np.log2(np.e)))
EXP16_B = float(np.float32(2**7 * (127.0 - EXP16_C)))
# exp cols on ACT (261ns fixed + 0.832ns/col = 1006ns); DVE takes the last
# 128 (~350ns).  NOTE the split axis is q-rows: rows 896..1023 of each
# core's 1024 get pure fast-exp (measured end-to-end 8.1e-3 vs gate 2e-2).
# 896 keeps ACT just under the ~1.09us/block PE floor; DVE's worst
# iteration (fast-exp + v-add) lands ~1.05us.
ACT_COLS = 896


def _install_drain_patch():
    """This compiler build caps sync-waits per instruction at 1; the Tile exit
    drain carries one wait per in-flight proc.  Split them across drains."""
    from bass_rust import ScopedClock

    if getattr(tile.TileContext, "_drain_patch_installed", False):
        return

    def _patched(self, tick_clock, wait_clock):
        drain_inst = self.nc.sync.drain()
        wait_clock.add_sem_waits(
            drain_inst.ins, ScopedClock({None: tick_clock.global_clock})
        )
        si = drain_inst.ins.sync_info
        waits = list(si.on_wait)
        last = drain_inst
        if len(waits) > 1:
            si.on_wait = waits[:1]
            for w in waits[1:]:
                extra = self.nc.sync.drain()
                extra.ins.sync_info = mybir.SyncInfo(on_wait=[w], on_update=[])
                last = extra
        assert self.sems is not None
        popped = self.nc._tile_sem_poison_stack.pop()
        assert popped is self._sem_poison
        sems = list(self.sems.allocated().values())
        if sems:
            nums = [s.num if hasattr(s, "num") else s for s in sems]
            span = range(min(nums), max(nums) + 1)
            # The drain chain above observed every proc's final tick, so all
            # sem consumers have retired; a single sem hop orders the clear
            # after it -- no all-engine barrier butterfly needed.
            gate = self.nc._state.alloc_semaphore()
            last.then_inc(gate, 1)
            self.nc.gpsimd.wait_ge(gate, 1)
            self.nc.gpsimd.sem_clear(span)
            self.nc.gpsimd.sem_clear(range(gate.num, gate.num + 1) if hasattr(gate, "num") else gate)

    tile.TileContext._drain_and_barrier = _patched
    tile.TileContext._drain_patch_installed = True


_CAP1_OPCODES = {"DMACopy", "Drain", "EventSemaphore", "TriggeredCopy"}
_DEFAULT_CAP = 1


def _legalize_waits(nc):
    """This toolchain encodes at most 1 sem-wait on queue/CTRL instructions
    (DMACopy, Drain) and ~2 on compute-engine instructions; Tile emits more.
    Hoist excess waits onto NoOp carriers on the same engine immediately
    before the overloaded instruction."""
    n_fix = 0
    for fn in nc.m.functions:
        for blk in fn.blocks:
            new_insts = []
            for inst in blk.instructions:
                si = inst.sync_info
                waits = list(si.on_wait) if si is not None else []
                cap = 1 if str(inst.opcode) in _CAP1_OPCODES else _DEFAULT_CAP
                if len(waits) > cap:
                    keep = waits[:cap]
                    rest = waits[cap:]
                    for k, w in enumerate(rest):
                        nop = mybir.InstNoOp(
                            name=f"{inst.name}-w{k}", ins=[], outs=[]
                        )
                        nop.engine = inst.engine
                        nop.sync_info = mybir.SyncInfo(on_wait=[w], on_update=[])
                        new_insts.append(nop)
                    inst.sync_info = mybir.SyncInfo(
                        on_wait=keep, on_update=list(si.on_update)
                    )
                    n_fix += 1
                new_insts.append(inst)
            blk.instructions = new_insts
    return n_fix


def build_bass():
    _install_drain_patch()
    nc = bass.Bass()
    xT = nc.dram_tensor("xT", [IN, N], BF, kind="ExternalInput")
    xTq = nc.dram_tensor("xTq", [IN, ROWS], BF, kind="ExternalInput")
    # Wall = Wq|Wk|Wv (two 128-row K-blocks each, side by side) followed by
    # the host-broadcast v-bias plane (every row = bv|bv|bv|bv).  The bias
    # plane is a full 128-partition block because 1-partition DMA completion
    # semaphores fire ~20us late on this runtime.
    Wall = nc.dram_tensor("Wall", [128, 3 * IN + 4 * OUT], BF, kind="ExternalInput")
    Ball = nc.dram_tensor("Ball", [128, 2], F32, kind="ExternalInput")
    out_d = nc.dram_tensor("out", [ROWS, OUT], F32, kind="ExternalOutput")

    AT = mybir.ActivationFunctionType
    OP = mybir.AluOpType

    NCHUNK = 16          # 512-token chunks
    BPC = 4              # tk blocks per chunk

    with tile.TileContext(nc) as tc:
        with (
            tc.tile_pool(name="persist", bufs=1) as persist,
            tc.tile_pool(name="wpool", bufs=1) as wpool,
            tc.tile_pool(name="pp", bufs=4) as pp,
            tc.tile_pool(name="ep", bufs=8) as ep,
            tc.tile_pool(name="pj", bufs=1, space="PSUM") as pj,
            tc.tile_pool(name="sp", bufs=2, space="PSUM") as sp,
            tc.tile_pool(name="avp", bufs=1, space="PSUM") as avp,
        ):
            # ---- persistent SBUF
            # x split into piece tiles so early chunks unblock as soon as
            # their piece lands (tile-granular deps; no subtile tracking).
            # each piece holds BOTH 128-row halves of xT side by side and is
            # filled by ONE 3D DMA -- fewer DMA rings means less per-queue
            # teardown churn in the walrus-generated postamble.
            PIECES = [(0, 512), (512, 1024), (1024, 2048), (2048, 4096), (4096, 6144), (6144, 8192)]
            xP = [persist.tile([128, 2 * (e - s0)], BF, tag=f"xP{i}", name=f"xP{i}")
                  for i, (s0, e) in enumerate(PIECES)]

            def xview(half, lo, hi):
                for i, (s0, e) in enumerate(PIECES):
                    if s0 <= lo and hi <= e:
                        w = e - s0
                        return xP[i][:, half * w + lo - s0 : half * w + hi - s0]
                raise AssertionError((lo, hi))

            def dma_piece(eng, i):
                s0, e = PIECES[i]
                dst = xP[i][:].rearrange("p (h c) -> p h c", h=2)
                src = xT[:, s0:e].rearrange("(h p) c -> p h c", p=128)
                eng.dma_start(dst, src)

            # xq in four [128,512] quarter tiles (two per 128-feature half):
            # the first q matmuls need only the 'a' quarters, so they start
            # ~1us into the xq transfer instead of after all 512KB
            xq0a = persist.tile([128, 512], BF, tag="xq0a")
            xq0b = persist.tile([128, 512], BF, tag="xq0b")
            xq1a = persist.tile([128, 512], BF, tag="xq1a")
            xq1b = persist.tile([128, 512], BF, tag="xq1b")
            kTs = [persist.tile([128, 512], BF, tag=f"kT{j}", name=f"kT{j}") for j in range(NCHUNK)]
            # chunk 0's kT additionally split per tk-block: S(0) then waits
            # only the first 128-row relu instead of the whole 512 (the
            # k-relu sits on the critical fill path to the first exp)
            kT0b = [persist.tile([128, 128], BF, tag=f"kT0b{t}", name=f"kT0b{t}")
                    for t in range(BPC)]
            vSs = [persist.tile([128, BPC * VW], BF, tag=f"vS{j}", name=f"vS{j}") for j in range(NCHUNK)]
            # qT in two half tiles: S(b)'s first matmul reads only half 0, so
            # it can issue after relu-0 instead of waiting for both q relus
            # (tile deps are whole-tile)
            qTh = [persist.tile([128, 512], BF, tag=f"qT{h}", name=f"qT{h}")
                   for h in range(2)]
            warm = persist.tile([128, 512], BF, tag="warm")

            wall = wpool.tile([128, 3 * IN + 4 * OUT], BF, tag="wall")
            ball = wpool.tile([128, 2], F32, tag="ball")
            bvb4 = wall[:, 3 * IN : 3 * IN + 4 * OUT]
            wq, wk, wv = wall[:, 0:IN], wall[:, IN : 2 * IN], wall[:, 2 * IN : 3 * IN]
            bq_s, bk_s = ball[:, 0:1], ball[:, 1:2]

            # ---- input DMA: only sync/scalar/gpsimd queues can trigger DMA.
            # The input path is AGGREGATE HBM-bandwidth bound (all 8 cores
            # pull the same data; the first-wave ~0.8MB/core window drains at
            # ~1.3TB/s total), so the wave is ordered by NEED: piece0+wk feed
            # the k0 projection (which overlaps the bigger xq wait), the xq
            # 'a' quarters feed the first q matmuls, 'b' quarters follow.
            dma_piece(nc.sync, 0)
            nc.sync.dma_start(xq0a[:], xTq[0:128, 0:512])
            nc.sync.dma_start(xq0b[:], xTq[0:128, 512:1024])
            nc.sync.dma_start(ball[:], Ball[:])

            # gpsimd: warm tile memset first (feeds the PE warmup below, and
            # must not wait behind the triggers' 0.7us each).
            nc.gpsimd.memset(warm[:], 0.0)
            nc.gpsimd.dma_start(wall[:, 0:IN], Wall[:, 0:IN])
            nc.gpsimd.dma_start(xq1a[:], xTq[128:256, 0:512])
            nc.gpsimd.dma_start(xq1b[:], xTq[128:256, 512:1024])

            # scalar (= ACT queue): weight transfers.  (No junk activation
            # needed: the auto-inserted ACT exp-table load has no deps and
            # runs right after these triggers, ~4us before the first q-relu.
            # Deferring wv+piece1 behind ACT-Copy spacers to clear the xq
            # HBM window was tried and measured ~0.8us WORSE on the fill:
            # the spacers push the table load into the q-relu chain.)
            nc.scalar.dma_start(wall[:, IN : 2 * IN], Wall[:, IN : 2 * IN])
            nc.scalar.dma_start(
                wall[:, 2 * IN : 3 * IN + 4 * OUT], Wall[:, 2 * IN : 3 * IN + 4 * OUT]
            )
            dma_piece(nc.scalar, 1)

            def ones_cols(jlo, jhi):
                for j in range(jlo, jhi):
                    vv = vSs[j][:].rearrange("p (b c) -> p b c", c=VW)
                    nc.gpsimd.memset(vv[:, :, OUT : OUT + 1], 1.0)

            # pieces 2-5 are spaced with dummy-memset timers so their
            # transfers start only after the critical window drains (v4
            # fired pieces 2-3 at ~9-11us and starved xq1/wq until ~17.7us:
            # 27MB of aggregate demand in an ~8MB/7us window).  Deadlines
            # (chunk c's k-proj at ~first_exp + 4.3c us) leave >2us margin.
            delay = wpool.tile([128, 3072], F32, tag="delay")
            ones_cols(0, NCHUNK)
            nc.gpsimd.memset(delay[:], 0.0)  # ~2.5us spacer
            dma_piece(nc.gpsimd, 2)
            nc.gpsimd.memset(delay[:], 0.0)
            dma_piece(nc.gpsimd, 3)
            nc.gpsimd.memset(delay[:], 0.0)
            dma_piece(nc.gpsimd, 4)
            nc.gpsimd.memset(delay[:], 0.0)
            dma_piece(nc.gpsimd, 5)

            # AV accumulators: 8 tq-chunks of [128, 129], 3 per PSUM bank pair
            av0 = avp.tile([128, 3 * VW], F32, tag="av0")
            av1 = avp.tile([128, 3 * VW], F32, tag="av1")
            av2 = avp.tile([128, 2 * VW], F32, tag="av2")
            chunk_map = [
                (av0, 0), (av0, 1), (av0, 2),
                (av1, 0), (av1, 1), (av1, 2),
                (av2, 0), (av2, 1),
            ]

            # PE warm-up burst off the memset tile: HAM clock-gate release +
            # DVFS ramp while the input DMAs are in flight (v3's warmups read
            # the wall tile and sat blocked on its DMA until ~9.7us).  3
            # matmuls measured best; stretching the burst to bridge the whole
            # DMA-wait window (9 matmuls) came back ~0.7us WORSE.
            for wu in range(3):
                nc.tensor.matmul(
                    av0[:, 0:384] if wu % 2 == 0 else av1[:, 0:384],
                    warm[:, 0:128],
                    warm[:, 128:512],
                    start=True, stop=True, skip_group_check=True,
                )

            # chunk-projection work in 2 sub-pieces staggered across the
            # chunk's iterations (the v3 schedule: it keeps TWO iterations
            # between each pj-bank release and the next alloc):
            #   t0: k matmuls + k-relu
            #   t2: v matmuls + v bias add + v max
            def proj_piece(j, t):
                if t == 0:
                    kp = pj.tile([128, 512], F32, tag="pj", name=f"kp_{j}")
                    nc.tensor.matmul(kp[:], wk[:, 0:128], xview(0, j * 512, (j + 1) * 512), start=True, stop=False)
                    nc.tensor.matmul(kp[:], wk[:, 128:256], xview(1, j * 512, (j + 1) * 512), start=False, stop=True)
                    if j == 0:
                        for tt in range(BPC):
                            nc.vector.tensor_scalar(
                                kT0b[tt][:], kp[:, tt * 128 : (tt + 1) * 128],
                                bk_s, 0.0, OP.add, OP.max,
                            )
                    else:
                        nc.vector.tensor_scalar(
                            kTs[j][:], kp[:], bk_s, 0.0, OP.add, OP.max
                        )
                elif t == 2:
                    vp = pj.tile([128, 512], F32, tag="pj", name=f"vp_{j}")
                    # all x0-side matmuls first, then all x1-side: each group
                    # shares ONE moving AP (a wv half) with rotating
                    # stationary slices -- the AV-stream pattern, which hides
                    # LDWEIGHTS.  Per-slice accumulation order is unchanged.
                    for vt in range(BPC):
                        lo = j * 512 + vt * 128
                        ds = slice(vt * 128, (vt + 1) * 128)
                        nc.tensor.matmul(
                            vp[:, ds], xview(0, lo, lo + 128), wv[:, 0:128],
                            start=(vt == 0), stop=False, skip_group_check=True,
                        )
                    for vt in range(BPC):
                        lo = j * 512 + vt * 128
                        ds = slice(vt * 128, (vt + 1) * 128)
                        nc.tensor.matmul(
                            vp[:, ds], xview(1, lo, lo + 128), wv[:, 128:256],
                            start=False, stop=(vt == BPC - 1), skip_group_check=True,
                        )
                    nc.vector.tensor_tensor(vp[:], vp[:], bvb4[:], mybir.AluOpType.add)
                    vview = vSs[j][:].rearrange("p (b c) -> p b c", c=VW)
                    vpview = vp[:].rearrange("p (b c) -> p b c", c=128)
                    nc.vector.tensor_scalar_max(vview[:, :, 0:OUT], vpview[:], 0.0)

            stiles = {}

            def emit_S(b):
                j, t = divmod(b, BPC)
                s = sp.tile([128, ROWS], F32, tag="s", name=f"s_{b}")
                lhs = kT0b[t][:] if j == 0 else kTs[j][:, t * 128 : (t + 1) * 128]
                nc.tensor.matmul(s[:, 0:512], lhs, qTh[0][:], start=True, stop=True)
                nc.tensor.matmul(
                    s[:, 512:1024], lhs, qTh[1][:], start=True, stop=True
                )
                stiles[b] = s

            ptiles = {}

            # exp is 100% ACT table-exp, as in v3.  Every DVE-assisted split
            # tried (v4-v8: trailing-cols Schraudolph, one-block-ahead
            # emission, separate output tiles) settled the steady cadence at
            # ~1.18us/block vs all-ACT's ~1.10: the extra cross-engine waits
            # and their NoOp carriers on the Tensor queue cost more than the
            # ~100ns/block of ACT relief buys.
            def emit_exp(b):
                s = stiles.pop(b)
                p = pp.tile([128, ROWS], BF, tag="p", name=f"p_{b}")
                ptiles[b] = p
                nc.scalar.activation(p[:], s[:], AT.Exp)

            def emit_AV(b):
                j, t = divmod(b, BPC)
                p = ptiles.pop(b)
                vblk = vSs[j][:, t * VW : (t + 1) * VW]
                # final block writes av2's chunks FIRST: the epilogue
                # processes av2 first, so its reciprocal unblocks ~0.3us
                # sooner (accumulation order within a block is free)
                order = reversed(range(8)) if b == NBLK - 1 else range(8)
                for c in order:
                    av, sub = chunk_map[c]
                    nc.tensor.matmul(
                        av[:, sub * VW : (sub + 1) * VW],
                        p[:, c * 128 : (c + 1) * 128],
                        vblk,
                        start=(b == 0 and sub == 0),
                        stop=(b == NBLK - 1),
                        skip_group_check=True,
                    )

            # Pipeline: the PE stream per iteration is [S(b+1); AV(b-1)],
            # with exp(b) split ACT/DVE in between.  S runs one block ahead
            # and AV one behind so the PE never waits on the exp of the block
            # it just produced -- keeps the tensor engine fed and at full
            # clock.  PSUM accumulation order across blocks is irrelevant
            # (block 0 carries start=, block 63 stop=).
            # ---- qT = relu(Wq.T @ xq + bq) ----
            # q-half 0 (from the xq 'a' quarters) then its relu; the chunk-0
            # k projection (matmuls + both DVE relu halves -- S(0) reads the
            # whole kT0 tile) goes between the two q halves so the k-chain
            # overlaps the 'b' quarters' transfer.  The two q halves use
            # SEPARATE psum tiles: with one shared tile, relu-0 blocks the
            # h1 matmuls on a whole-tile write-after-read for ~2us.
            # q relus ride ACT (idle until exp(0); Relu shares the exp table
            # set so no table reload) -- keeps the DVE queue off the S(0)
            # critical chain.
            # chunk-0 k chain FIRST in the PE stream: piece0 and wk are the
            # first transfers on their queues, so the kp matmuls + DVE relu
            # run while the (bigger) xq quarters are still in flight.
            proj_piece(0, 0)
            for h, (xa, xb) in enumerate(((xq0a, xq1a), (xq0b, xq1b))):
                qp = sp.tile([128, 512], F32, tag="s", name=f"qp{h}")
                nc.tensor.matmul(qp[:], wq[:, 0:128], xa[:],
                                 start=True, stop=False, skip_group_check=True)
                nc.tensor.matmul(qp[:], wq[:, 128:256], xb[:],
                                 start=False, stop=True, skip_group_check=True)
                nc.scalar.activation(qTh[h][:], qp[:], AT.Relu, bias=bq_s)

            # v0 comes AFTER S(0): the first exp only needs q/k0/S0, and v0
            # is needed one block later by AV(0)
            emit_S(0)
            avq = []
            for j in range(NCHUNK):
                for t in range(BPC):
                    b = j * BPC + t
                    if b + 1 < NBLK:
                        emit_S(b + 1)
                    if b == 0:
                        # chunk-0 v chain AFTER S(1) in the PE stream: its
                        # 8 matmuls otherwise delay exp(1) by ~1us during
                        # pipeline fill (AV(0), their consumer, runs a full
                        # block later and is not critical)
                        proj_piece(0, 2)
                    emit_exp(b)
                    # stagger next chunk's projection pieces between blocks;
                    # k at t==0 / v at t==2 gives the DVE relu chain a full
                    # extra block of slack before kT/vS are consumed at the
                    # chunk boundary
                    if j + 1 < NCHUNK:
                        if t == 0:
                            proj_piece(j + 1, 0)
                        elif t == 2:
                            proj_piece(j + 1, 2)
                    avq.append(b)
                    while avq and avq[0] <= b - 1:
                        emit_AV(avq.pop(0))
            for b in avq:
                emit_AV(b)

            # ---- epilogue: divide by the ones-column denominator, DMA out.
            # one strided reciprocal + grouped result tile + one DMA per av
            # accumulator, the three DMAs on three DIFFERENT queues so their
            # trigger cost and completion-semaphore latency overlap.  av2
            # first: the final AV block writes its chunks first, so its
            # reciprocal unblocks soonest.
            # each av's multiplies are SINGLE-engine (av2/av0 on DVE, av1 on
            # ACT) so no av's DMA trigger chains behind another av's ops on a
            # shared engine queue (v5's av1 trigger on the scalar queue sat
            # behind all three ACT copies).  av0 -- the last one ready --
            # rides sync as its SECOND trigger: the gpsimd DMA ring's
            # completion semaphore lags ~1us behind the sync/scalar rings,
            # and av0's completion gates the exit drain.
            for av, nsub, base, eng, mul_eng in (
                (av2, 2, 6, nc.sync, "dve"),
                (av1, 3, 3, nc.scalar, "act"),
                (av0, 3, 0, nc.sync, "dve"),
            ):
                avv = av[:].rearrange("p (b c) -> p b c", c=VW)
                rc = ep.tile([128, nsub], F32, tag=f"rc{base}", name=f"rc_{base}")
                nc.vector.reciprocal(rc[:], avv[:, :, OUT])
                res = ep.tile([128, nsub * OUT], F32, tag=f"res{base}", name=f"res_{base}")
                rview = res[:].rearrange("p (b c) -> p b c", c=OUT)
                for s2 in range(nsub):
                    if mul_eng == "dve":
                        nc.vector.tensor_scalar_mul(
                            rview[:, s2, :], avv[:, s2, 0:OUT], rc[:, s2 : s2 + 1]
                        )
                    else:
                        nc.scalar.activation(
                            rview[:, s2, :], avv[:, s2, 0:OUT], AT.Copy,
                            scale=rc[:, s2 : s2 + 1],
                        )
                dst = out_d[base * 128 : (base + nsub) * 128, :].rearrange(
                    "(b p) c -> p b c", p=128
                )
                eng.dma_start(dst, rview[:])

    _legalize_waits(nc)
    return nc


_NC_CACHE = None


def _get_nc():
    global _NC_CACHE
    if _NC_CACHE is None:
        _NC_CACHE = build_bass()
    return _NC_CACHE


def _prep_inputs(x, Wq, bq, Wk, bk, Wv, bv):
    bf = ml_dtypes.bfloat16
    xT = np.ascontiguousarray(np.asarray(x, np.float32).T).astype(bf)  # [256, 8192]

    def w2(W):  # [256,128] -> [128, 256] with the two 128-row K-blocks side by side
        W = np.asarray(W, np.float32)
        return np.ascontiguousarray(np.concatenate([W[:128], W[128:]], axis=1)).astype(bf)

    base = {
        "xT": xT,
        "Wall": np.ascontiguousarray(
            np.concatenate(
                [
                    w2(Wq), w2(Wk), w2(Wv),
                    np.broadcast_to(
                        np.tile(np.asarray(bv, np.float32), 4)[None, :],
                        (128, 4 * OUT),
                    ).astype(bf),
                ],
                axis=1,
            )
        ),
        "Ball": np.ascontiguousarray(
            np.stack(
                [np.asarray(bq, np.float32), np.asarray(bk, np.float32)], axis=1
            )
        ),
    }
    in_maps = []
    for c in range(NCORES):
        m = dict(base)
        m["xTq"] = np.ascontiguousarray(xT[:, c * ROWS : (c + 1) * ROWS])
        in_maps.append(m)
    return in_maps


def kernel(x, Wq, bq, Wk, bk, Wv, bv):
    nc = _get_nc()
    in_maps = _prep_inputs(x, Wq, bq, Wk, bk, Wv, bv)
    last_err = None
    for attempt in range(3):
        try:
            res = run_bass_kernel_spmd(nc, in_maps, core_ids=list(range(NCORES)))
            break
        except Exception as e:  # transient NRT_EXEC_UNIT_UNRECOVERABLE after a
            last_err = e       # previously crashed run wedges the device once
            if attempt == 2:
                raise
            time.sleep(2)
    return np.concatenate([res.results[c]["out"] for c in range(NCORES)], axis=0)


if __name__ == "__main__":
    rng = np.random.default_rng(0)
    s = 1.0 / np.sqrt(IN)
    x = rng.standard_normal((N, IN), dtype=np.float32)
    args = dict(
        x=x,
        Wq=rng.uniform(-s, s, (IN, OUT)).astype(np.float32),
        bq=rng.uniform(-s, s, OUT).astype(np.float32),
        Wk=rng.uniform(-s, s, (IN, OUT)).astype(np.float32),
        bk=rng.uniform(-s, s, OUT).astype(np.float32),
        Wv=rng.uniform(-s, s, (IN, OUT)).astype(np.float32),
        bv=rng.uniform(-s, s, OUT).astype(np.float32),
    )
    o = kernel(**args)
    q = np.maximum(x @ args["Wq"] + args["bq"], 0)
    k = np.maximum(x @ args["Wk"] + args["bk"], 0)
    v = np.maximum(x @ args["Wv"] + args["bv"], 0)
    S = q @ k.T
    P = np.exp(S - S.max(1, keepdims=True))
    ref = (P / P.sum(1, keepdims=True)) @ v
    print("max rel err:", np.abs(o - ref).max() / np.abs(ref).max())
